# revision 17
# baseline (speedup 1.0000x reference)
"""LogitSeparator Trainium2 kernel.

For each (b, d) of schemas (64, 32), left-align the zone
logits[b, start:end] (length = schemas[b,d] <= 255) into out[b, d, :8192],
zero padded, plus a boolean in-zone mask.

Strategy: pure data parallel over the batch dim (8 rows per core).  Per
core the 256 ragged (b, d) rows map onto 2 x 128 SBUF partitions.  Two
indirect DMAs (one per half; HW reads one offset per partition) gather
each row's 256-element slab from the (padded, flat) logits in DRAM.  The
vector engine builds the j < len mask (u8 for the mask output, f32 to
zero the slab tail garbage in a per-half mult), and the slabs ship on
the two HWDGE rings (SP: out half 0 + half 1a; ACT: mask, then half 1b)
as each half's mult lands.  Each gather call carries its own
completion semaphore: the 16 per-engine increments of a shared sem can
mix across calls, releasing a half's mult while a slow engine's data
for that half is still in flight (seen as engine-aligned groups of
un-zeroed slab tails).  The out/mask tails [256:8192] are never written:
the runtime zero-fills ExternalOutput DRAM buffers on both execution
paths (native run_bass_kernel_spmd pre-zeros them; the axon/PJRT path
donates np.zeros buffers), so the tails are already correct.
"""

import ml_dtypes
import numpy as np

import concourse.bass as bass
import concourse.mybir as mybir
from concourse.bass_utils import run_bass_kernel_spmd

B, D, L = 64, 32, 8192
NCORES = 8
BPC = B // NCORES           # batch rows per core
R = BPC * D                 # ragged rows per core (256)
P = 128                     # SBUF partitions
HALVES = R // P             # 2
SLAB = 256                  # max zone length (schemas < 256)
W = HALVES * SLAB           # 512
NPAD = BPC * L + SLAB       # padded flat logits length per core

_NC_CACHE = {}


# aux layout (int32): cols [0:2] gather flat-start idx per half, col [2]
# the two zone lens per half packed as an int16 pair.
AUXW = HALVES + 1


def build_nc():
    nc = bass.Bass()
    lg = nc.declare_dram_parameter(
        "logits_flat", [NPAD, 1], mybir.dt.bfloat16, isOutput=False
    )
    aux = nc.declare_dram_parameter("aux", [P, AUXW], mybir.dt.int32, isOutput=False)
    out = nc.declare_dram_parameter("out", [R, L], mybir.dt.float32, isOutput=True)
    msk = nc.declare_dram_parameter("mask", [R, L], mybir.dt.uint8, isOutput=True)

    msk3 = msk.rearrange("(h p) l -> p h l", p=P)  # row r = h*128+p <- [p,h,:]
    with (
        nc.sbuf_tensor([P, AUXW], mybir.dt.int32) as aux_t,
        nc.sbuf_tensor([P, SLAB], mybir.dt.int16) as iota_t,
        nc.sbuf_tensor([16, AUXW], mybir.dt.int32) as kick_t,
        nc.sbuf_tensor([P, W], mybir.dt.bfloat16) as gatb,
        nc.sbuf_tensor([P, W], mybir.dt.float32) as gato,
        nc.sbuf_tensor([P, W], mybir.dt.bfloat16) as maskb,
        nc.sbuf_tensor([P, W], mybir.dt.uint8) as masku2,
        nc.semaphore("asem") as asem,  # aux input DMA completion
        nc.semaphore("g0sem") as g0sem,  # gather half-0 completion
        nc.semaphore("g1sem") as g1sem,  # gather half-1 completion
        nc.semaphore("isem") as isem,  # iota ready
        nc.semaphore("vsem") as vsem,  # DVE milestones
        nc.semaphore("dsem") as dsem,  # output DMA completions
        nc.semaphore("ksem") as ksem,  # doorbell-kick DMA completion
        nc.Block(no_gpsimd_drain=True) as block,
    ):
        iota_b = iota_t[:].unsqueeze(1).to_broadcast([P, HALVES, SLAB])
        lens_b = (
            aux_t[:, HALVES : HALVES + 1]
            .bitcast(mybir.dt.int16)
            .unsqueeze(2)
            .to_broadcast([P, HALVES, SLAB])
        )

        @block.sync
        def _(sync):
            sync.dma_start(out=aux_t[:], in_=aux[:]).then_inc(asem, 16)
            # Half 0 of out ships as soon as its mult lands (vsem >= 2).
            sync.wait_ge(vsem, 2)
            sync.dma_start(
                out=out[0:P, 0:SLAB], in_=gato[:, 0:SLAB]
            ).then_inc(dsem, 16)
            # Half 1a on the SP ring (even SDMA engines).
            sync.wait_ge(vsem, 3)
            sync.dma_start(
                out=out[P : P + 64, 0:SLAB], in_=gato[0:64, SLAB:W]
            ).then_inc(dsem, 16)
            # All four output DMAs landed before the kernel ends.
            sync.wait_ge(dsem, 64)

        @block.scalar
        def _(sc):
            # Mask slab only needs the u8 is_lt (vsem >= 1); ACT is the
            # second HWDGE ring, so this overlaps the SP-ring traffic.
            sc.wait_ge(vsem, 1)
            sc.dma_start(
                out=msk3[:, :, 0:SLAB],
                in_=masku2[:].rearrange("p (h j) -> p h j", h=HALVES),
            ).then_inc(dsem, 16)
            # Half 1b on the ACT ring (odd SDMA engines).
            sc.wait_ge(vsem, 3)
            sc.dma_start(
                out=out[P + 64 : 2 * P, 0:SLAB], in_=gato[64:P, SLAB:W]
            ).then_inc(dsem, 16)

        @block.gpsimd
        def _(gp):
            gp.iota(
                iota_t[:], pattern=[[1, SLAB]], base=0, channel_multiplier=0
            ).then_inc(isem, 1)
            gp.wait_ge(asem, 16)  # gather offsets in SBUF
            # One indirect gather per half (HW reads one offset per
            # partition): offset (p, h) feeds gat2[p, h*SLAB:(h+1)*SLAB].
            # Each call gets its OWN semaphore: the 16 per-engine incs of
            # a shared sem could mix across calls, firing a half's wait
            # while a slow engine's data for that half is still in
            # flight.
            for h, gs in ((0, g0sem), (1, g1sem)):
                gp.indirect_dma_start(
                    out=gatb[:, h * SLAB : (h + 1) * SLAB],
                    out_offset=None,
                    in_=lg[:],
                    in_offset=bass.IndirectOffsetOnAxis(
                        ap=aux_t[:, h : h + 1], axis=0
                    ),
                ).then_inc(gs, 16)
            # Extra SWDGE doorbell right after the gathers: insurance
            # against the intermittent SDMA-engine stall where one
            # engine parks for ~3us with descriptors pending.
            gp.dma_start(out=kick_t[:], in_=aux[0:16, :]).then_inc(ksem, 16)
            gp.wait_ge(ksem, 16)  # retire the kick before teardown

        @block.vector
        def _(v):
            v.wait_ge(isem, 1)   # iota in SBUF
            v.wait_ge(asem, 16)  # zone lens in SBUF
            # mask[p, h, j] = j < len_ph, as u8 for the mask output and
            # f32 for the slab-tail zeroing mults.
            v.tensor_tensor(
                out=masku2[:].rearrange("p (h j) -> p h j", h=HALVES),
                in0=iota_b,
                in1=lens_b,
                op=mybir.AluOpType.is_lt,
            ).then_inc(vsem, 1)
            v.tensor_tensor(
                out=maskb[:].rearrange("p (h j) -> p h j", h=HALVES),
                in0=iota_b,
                in1=lens_b,
                op=mybir.AluOpType.is_lt,
            )
            v.drain()  # flush DVE pipeline: maskb RAW in the mults below
            # Zero the gathered tail garbage (j >= len) per half as each
            # half's gather fully lands.
            v.wait_ge(g0sem, 16)
            v.tensor_mul(
                out=gato[:, 0:SLAB], in0=gatb[:, 0:SLAB], in1=maskb[:, 0:SLAB]
            ).then_inc(vsem, 1)
            v.wait_ge(g1sem, 16)
            v.tensor_mul(
                out=gato[:, SLAB:W], in0=gatb[:, SLAB:W], in1=maskb[:, SLAB:W]
            ).then_inc(vsem, 1)
    return nc


def _get_nc():
    if "nc" not in _NC_CACHE:
        _NC_CACHE["nc"] = build_nc()
    return _NC_CACHE["nc"]


def make_in_maps(schemas, logits):
    """Shard full inputs into per-core input maps for the SPMD kernel."""
    sch = np.asarray(schemas).astype(np.int64)
    lg = np.ascontiguousarray(np.asarray(logits, dtype=np.float32))
    cs = np.cumsum(sch, axis=1)
    start = cs - sch                     # (B, D) zone starts
    ln = sch.astype(np.int32)            # (B, D) zone lengths

    in_maps = []
    for c in range(NCORES):
        b0 = c * BPC
        flat = np.concatenate(
            [lg[b0 : b0 + BPC].reshape(-1), np.zeros(SLAB, np.float32)]
        ).astype(ml_dtypes.bfloat16).reshape(NPAD, 1)
        gflat = (
            np.arange(BPC, dtype=np.int64)[:, None] * L + start[b0 : b0 + BPC]
        ).reshape(R)
        lnc = ln[b0 : b0 + BPC].reshape(R).reshape(HALVES, P).T  # [P, HALVES]
        aux = np.empty((P, AUXW), dtype=np.int32)
        # row r = h*128 + p  ->  aux[p, h]
        aux[:, 0:HALVES] = gflat.reshape(HALVES, P).T
        # lens as a packed little-endian int16 pair in col HALVES
        aux[:, HALVES] = (lnc[:, 0] | (lnc[:, 1] << 16)).astype(np.int32)
        in_maps.append({"logits_flat": flat, "aux": aux})
    return in_maps


def assemble(results):
    """Gather per-core outputs back into full-shape arrays."""
    out = np.concatenate(
        [np.asarray(results[c]["out"]).reshape(BPC, D, L) for c in range(NCORES)],
        axis=0,
    )
    msk = np.concatenate(
        [np.asarray(results[c]["mask"]).reshape(BPC, D, L) for c in range(NCORES)],
        axis=0,
    )
    if msk.dtype != np.bool_:
        msk = msk.astype(np.uint8).view(np.bool_)
    return out, msk


def kernel(schemas, logits):
    in_maps = make_in_maps(schemas, logits)
    nc = _get_nc()
    res = run_bass_kernel_spmd(nc, in_maps, list(range(NCORES))).results
    return assemble(res)


# revision 18
# speedup vs baseline: 1.0154x; 1.0154x over previous
"""LogitSeparator Trainium2 kernel.

For each (b, d) of schemas (64, 32), left-align the zone
logits[b, start:end] (length = schemas[b,d] <= 255) into out[b, d, :8192],
zero padded, plus a boolean in-zone mask.

Strategy: pure data parallel over the batch dim (8 rows per core).  Per
core the 256 ragged (b, d) rows map onto 2 x 128 SBUF partitions.  Two
indirect DMAs (one per half; HW reads one offset per partition) gather
each row's 256-element slab from the (padded, flat) logits in DRAM.  The
vector engine builds the j < len mask (u8 for the mask output, f32 to
zero the slab tail garbage in a per-half mult), and the slabs ship on
the two HWDGE rings (SP: out half 0 + half 1a; ACT: mask, then half 1b)
as each half's mult lands.  Each gather call carries its own
completion semaphore: the 16 per-engine increments of a shared sem can
mix across calls, releasing a half's mult while a slow engine's data
for that half is still in flight (seen as engine-aligned groups of
un-zeroed slab tails).  The out/mask tails [256:8192] are never written:
the runtime zero-fills ExternalOutput DRAM buffers on both execution
paths (native run_bass_kernel_spmd pre-zeros them; the axon/PJRT path
donates np.zeros buffers), so the tails are already correct.
"""

import ml_dtypes
import numpy as np

import concourse.bass as bass
import concourse.mybir as mybir
from concourse.bass_utils import run_bass_kernel_spmd

B, D, L = 64, 32, 8192
NCORES = 8
BPC = B // NCORES           # batch rows per core
R = BPC * D                 # ragged rows per core (256)
P = 128                     # SBUF partitions
HALVES = R // P             # 2
SLAB = 256                  # max zone length (schemas < 256)
W = HALVES * SLAB           # 512
NPAD = BPC * L + SLAB       # padded flat logits length per core

_NC_CACHE = {}


# aux layout (int32): cols [0:2] gather flat-start idx per half, col [2]
# the two zone lens per half packed as an int16 pair.
AUXW = HALVES + 1


def build_nc():
    nc = bass.Bass()
    lg = nc.declare_dram_parameter(
        "logits_flat", [NPAD, 1], mybir.dt.bfloat16, isOutput=False
    )
    aux = nc.declare_dram_parameter("aux", [P, AUXW], mybir.dt.int32, isOutput=False)
    out = nc.declare_dram_parameter("out", [R, L], mybir.dt.float32, isOutput=True)
    msk = nc.declare_dram_parameter("mask", [R, L], mybir.dt.uint8, isOutput=True)

    msk3 = msk.rearrange("(h p) l -> p h l", p=P)  # row r = h*128+p <- [p,h,:]
    with (
        nc.sbuf_tensor([P, AUXW], mybir.dt.int32) as aux_t,
        nc.sbuf_tensor([P, SLAB], mybir.dt.int16) as iota_t,
        nc.sbuf_tensor([16, AUXW], mybir.dt.int32) as kick_t,
        nc.sbuf_tensor([P, W], mybir.dt.bfloat16) as gatb,
        nc.sbuf_tensor([P, W], mybir.dt.float32) as gato,
        nc.sbuf_tensor([P, W], mybir.dt.bfloat16) as maskb,
        nc.sbuf_tensor([P, W], mybir.dt.uint8) as masku2,
        nc.semaphore("asem") as asem,  # aux input DMA completion
        nc.semaphore("g0sem") as g0sem,  # gather half-0 completion
        nc.semaphore("g1sem") as g1sem,  # gather half-1 completion
        nc.semaphore("isem") as isem,  # iota ready
        nc.semaphore("vsem") as vsem,  # DVE milestones
        nc.semaphore("dsem") as dsem,  # output DMA completions
        nc.semaphore("ksem") as ksem,  # doorbell-kick DMA completion
        nc.Block(no_gpsimd_drain=True) as block,
    ):
        iota_b = iota_t[:].unsqueeze(1).to_broadcast([P, HALVES, SLAB])
        lens_b = (
            aux_t[:, HALVES : HALVES + 1]
            .bitcast(mybir.dt.int16)
            .unsqueeze(2)
            .to_broadcast([P, HALVES, SLAB])
        )

        @block.sync
        def _(sync):
            sync.dma_start(out=aux_t[:], in_=aux[:]).then_inc(asem, 16)
            # Half 0 of out ships as soon as its mult lands (vsem >= 2).
            sync.wait_ge(vsem, 2)
            sync.dma_start(
                out=out[0:P, 0:SLAB], in_=gato[:, 0:SLAB]
            ).then_inc(dsem, 16)
            # Half 1a on the SP ring (even SDMA engines).
            sync.wait_ge(vsem, 3)
            sync.dma_start(
                out=out[P : P + 64, 0:SLAB], in_=gato[0:64, SLAB:W]
            ).then_inc(dsem, 16)
            # All four output DMAs landed before the kernel ends.
            sync.wait_ge(dsem, 64)

        @block.scalar
        def _(sc):
            # Mask slab only needs the u8 is_lt (vsem >= 1); ACT is the
            # second HWDGE ring, so this overlaps the SP-ring traffic.
            sc.wait_ge(vsem, 1)
            sc.dma_start(
                out=msk3[:, :, 0:SLAB],
                in_=masku2[:].rearrange("p (h j) -> p h j", h=HALVES),
            ).then_inc(dsem, 16)
            # Half 1b on the ACT ring (odd SDMA engines).
            sc.wait_ge(vsem, 3)
            sc.dma_start(
                out=out[P + 64 : 2 * P, 0:SLAB], in_=gato[64:P, SLAB:W]
            ).then_inc(dsem, 16)

        @block.gpsimd
        def _(gp):
            gp.iota(
                iota_t[:], pattern=[[1, SLAB]], base=0, channel_multiplier=0
            ).then_inc(isem, 1)
            gp.wait_ge(asem, 16)  # gather offsets in SBUF
            # Tiny iota between the blocked wait and the first indirect:
            # an indirect dispatched straight after a blocked sem-wait
            # pays ~1us of wakeup, but compute ops resume in ~0.1us and
            # a subsequent indirect then dispatches in ~0.3us.
            gp.iota(kick_t[0:2, 0:1], pattern=[[1, 1]], base=0, channel_multiplier=0)
            # One indirect gather per half (HW reads one offset per
            # partition): offset (p, h) feeds gat2[p, h*SLAB:(h+1)*SLAB].
            # Each call gets its OWN semaphore: the 16 per-engine incs of
            # a shared sem could mix across calls, firing a half's wait
            # while a slow engine's data for that half is still in
            # flight.
            for h, gs in ((0, g0sem), (1, g1sem)):
                gp.indirect_dma_start(
                    out=gatb[:, h * SLAB : (h + 1) * SLAB],
                    out_offset=None,
                    in_=lg[:],
                    in_offset=bass.IndirectOffsetOnAxis(
                        ap=aux_t[:, h : h + 1], axis=0
                    ),
                ).then_inc(gs, 16)
            # Extra SWDGE doorbell right after the gathers: insurance
            # against the intermittent SDMA-engine stall where one
            # engine parks for ~3us with descriptors pending.
            gp.dma_start(out=kick_t[:], in_=aux[0:16, :]).then_inc(ksem, 16)
            gp.wait_ge(ksem, 16)  # retire the kick before teardown

        @block.vector
        def _(v):
            v.wait_ge(isem, 1)   # iota in SBUF
            v.wait_ge(asem, 16)  # zone lens in SBUF
            # mask[p, h, j] = j < len_ph, as u8 for the mask output and
            # f32 for the slab-tail zeroing mults.
            v.tensor_tensor(
                out=masku2[:].rearrange("p (h j) -> p h j", h=HALVES),
                in0=iota_b,
                in1=lens_b,
                op=mybir.AluOpType.is_lt,
            ).then_inc(vsem, 1)
            v.tensor_tensor(
                out=maskb[:].rearrange("p (h j) -> p h j", h=HALVES),
                in0=iota_b,
                in1=lens_b,
                op=mybir.AluOpType.is_lt,
            )
            v.drain()  # flush DVE pipeline: maskb RAW in the mults below
            # Zero the gathered tail garbage (j >= len) per half as each
            # half's gather fully lands.
            v.wait_ge(g0sem, 16)
            v.tensor_mul(
                out=gato[:, 0:SLAB], in0=gatb[:, 0:SLAB], in1=maskb[:, 0:SLAB]
            ).then_inc(vsem, 1)
            v.wait_ge(g1sem, 16)
            v.tensor_mul(
                out=gato[:, SLAB:W], in0=gatb[:, SLAB:W], in1=maskb[:, SLAB:W]
            ).then_inc(vsem, 1)
    return nc


def _get_nc():
    if "nc" not in _NC_CACHE:
        _NC_CACHE["nc"] = build_nc()
    return _NC_CACHE["nc"]


def make_in_maps(schemas, logits):
    """Shard full inputs into per-core input maps for the SPMD kernel."""
    sch = np.asarray(schemas).astype(np.int64)
    lg = np.ascontiguousarray(np.asarray(logits, dtype=np.float32))
    cs = np.cumsum(sch, axis=1)
    start = cs - sch                     # (B, D) zone starts
    ln = sch.astype(np.int32)            # (B, D) zone lengths

    in_maps = []
    for c in range(NCORES):
        b0 = c * BPC
        flat = np.concatenate(
            [lg[b0 : b0 + BPC].reshape(-1), np.zeros(SLAB, np.float32)]
        ).astype(ml_dtypes.bfloat16).reshape(NPAD, 1)
        gflat = (
            np.arange(BPC, dtype=np.int64)[:, None] * L + start[b0 : b0 + BPC]
        ).reshape(R)
        lnc = ln[b0 : b0 + BPC].reshape(R).reshape(HALVES, P).T  # [P, HALVES]
        aux = np.empty((P, AUXW), dtype=np.int32)
        # row r = h*128 + p  ->  aux[p, h]
        aux[:, 0:HALVES] = gflat.reshape(HALVES, P).T
        # lens as a packed little-endian int16 pair in col HALVES
        aux[:, HALVES] = (lnc[:, 0] | (lnc[:, 1] << 16)).astype(np.int32)
        in_maps.append({"logits_flat": flat, "aux": aux})
    return in_maps


def assemble(results):
    """Gather per-core outputs back into full-shape arrays."""
    out = np.concatenate(
        [np.asarray(results[c]["out"]).reshape(BPC, D, L) for c in range(NCORES)],
        axis=0,
    )
    msk = np.concatenate(
        [np.asarray(results[c]["mask"]).reshape(BPC, D, L) for c in range(NCORES)],
        axis=0,
    )
    if msk.dtype != np.bool_:
        msk = msk.astype(np.uint8).view(np.bool_)
    return out, msk


def kernel(schemas, logits):
    in_maps = make_in_maps(schemas, logits)
    nc = _get_nc()
    res = run_bass_kernel_spmd(nc, in_maps, list(range(NCORES))).results
    return assemble(res)
